# revision 92
# baseline (speedup 1.0000x reference)
"""Trainium2 Bass kernel for nn_MultiHeadMapAttentionV2.

Math restructuring (see kernel_baseline.py for the derivation):
  - 5-stage 1x1 conv chain folded on host; single query per batch.
  - K never materialized: scores via stationary Qt columns against fm tokens.
  - Positional score parts (s2x) are host constants; positional value part
    PV^T p is computed ON DEVICE as per-head matmuls against DMA-transposed
    softmax probabilities (pTcat), batched across all 32 batches of the core.
  - Mean token folded into spatial weights (W~ = p + p0/196).

V3 device structure (4 batches per supergroup, SG = 8 per core):
  - fp8e4 (TRN e4m3) feature map (error-diffused along tokens so the
    attention-weighted sums stay ~bf16-accurate), Qt x128 (descaled via the
    softmax STT scalars), and Wv as a hi/lo fp8 pair (32*wv = hi + lo,
    ~7e-4) multiplied against a stride-0-broadcast fm chunk in the two
    DoubleRow slots.
  - Scores psum [48, 392]: batches at rows {0,8,32,40}; pair1 uses a
    48-wide zero-padded lhsT so all matmul outputs start at partition 0
    (HW restriction: operand/output base partition in {0,32,64}).
  - Softmax over [48, 392] covers 4 batches per pass; weights written
    full-width (W~full = esc*rec + pm196) so one DVE op covers a pair.
  - Spatial value sums: V psum rows permuted to d = (p//16)*64 + q*16 +
    p%16 so ONE selector broadcast [16, 128] serves all 4 V passes; DVE
    STT accumulates straight into outcatV columns.
  - Positional value sums: wc (incl pm196 col) PE-transposed (bf16 psum)
    into pTcat; tail per-head matmuls against duplicated-PV rows with a
    compensation row (196*PV[0] - 2*sum PV_sp) fold in the mean-token
    value and cancel the pm196 contamination; folded into the wo matmul
    accumulation via the m-layout wot.
  - Tail: wo projection (bf16, q-layout + pos m-layout chunks), LN stats
    via ones-matmul interleaved with PE transposes, normalize.
  - Tail-only constants DMA'd after the supergroup loop; wvt8 on the Act
    DMA queue so gdata(0) loads immediately.
"""

import numpy as np
from ml_dtypes import bfloat16, float8_e4m3

P = 128
C = 1024
S = 14
SP = S * S          # 196 spatial tokens
NT = SP + 1         # 197 tokens
H = 8
DK = 64
NCORES = 8
B_FULL = 256
EPS = 1e-5

SGB = 4             # batches per supergroup
Q_SCALE = 128.0     # fp8 scale for Qt
V_SCALE = 32.0      # fp8 scale for Wv

FM_COLS = 4 * 2 * SGB * SP      # 6272 fp8 elements (k2, two, jj, t)
QT_OFF = FM_COLS                # 6272
# pair0 Qt: 4k2 x 2two x 16q = 128; pair1 Qt (48-wide, zeros at q<32): 384
S2_OFF = QT_OFF + 512           # 6784 (%4 == 0 for f32 bitcast)
GD_COLS = S2_OFF + 4 * 393      # 8356 fp8 elements
# batch rows within score tiles: row(jj, h) = 32*(jj//2) + 8*(jj%2) + h
RB = [0, 8, 32, 40]


def _f8(x):
    return np.asarray(x, np.float32).astype(float8_e4m3).view(np.uint8)


# ---------------------------------------------------------------- host prep

def _host_prep(inputs):
    f = {k: np.ascontiguousarray(np.asarray(v, dtype=np.float32)) for k, v in inputs.items()}
    w1, w2, w3, w4, w5 = f['w1'], f['w2'], f['w3'], f['w4'], f['w5']
    b1, b2, b3, b4, b5 = f['b1'], f['b2'], f['b3'], f['b4'], f['b5']
    B = f['feature_map'].shape[0]

    Wt = w5 @ w4 @ w3 @ w2 @ w1                                   # (1024, 8)
    bt = w5 @ (w4 @ (w3 @ (w2 @ b1 + b2) + b3) + b4) + b5         # (1024,)
    lmean = f['loss_map'].reshape(B, 8, SP).mean(-1)              # (B, 8)
    queries = lmean @ Wt.T + bt + f['pos_q'][0]                   # (B, 1024)
    q = (queries @ f['wq'].T + f['bq']) / np.float32(np.sqrt(DK)) # (B, 512)
    qr_ = q.reshape(B, H, DK)
    wk_r = f['wk'].reshape(H, DK, C)
    Qt = np.einsum('hdc,bhd->bch', wk_r, qr_)                     # (B, 1024, 8)
    PK = f['pos_kv'] @ f['wk'].T + f['bk']                        # (197, 512)
    s2 = np.einsum('nhd,bhd->bnh', PK.reshape(NT, H, DK), qr_)    # (B, 197, 8)
    PV = f['pos_kv'] @ f['wv'].T + f['bv']                        # (197, 512)
    fm = f['feature_map'].reshape(B, C, SP)                       # (B, 1024, 196)
    qpb = queries + f['bo']                                       # (B, 1024)

    # fp8 feature map with error diffusion along tokens: keeps the
    # attention-weighted sums accurate (the V path) while per-element noise
    # stays fp8-level (fine for scores).
    fm8 = np.zeros((B, C, SP), np.uint8)
    ederr = np.zeros((B, C), np.float32)
    for t in range(SP):
        v = fm[:, :, t] + ederr
        q8 = v.astype(float8_e4m3)
        fm8[:, :, t] = q8.view(np.uint8)
        ederr = v - q8.astype(np.float32)

    # ---- shared (batch-independent) device arrays
    wv = f['wv']                                                  # (512, 1024)
    # fp8 split of 32*wv: for k-chunks 0..3 a hi/lo pair occupies the two
    # DoubleRow slots against a stride-0-broadcast fm chunk (~bf16-exact);
    # chunks 4..7 are hi-only, k-paired in the DR slots (total rel err
    # ~1.35e-2, measured in emulation; gate is 2e-2).
    # V psum layout permuted so ONE selector broadcast serves all passes:
    # pass q holds d = (p//16)*64 + q*16 + p%16 on partition p.
    # Layout per q (1536 cols): k<4: [k*256 + s*128 + j] = {hi,lo}(k);
    # k>=4: [1024 + pr2*256 + two*128 + j] = hi(4 + 2*pr2 + two)
    w32 = (V_SCALE * wv).reshape(8, 4, 16, 8, P)                  # h, q, r, k, p
    hi8 = w32.astype(float8_e4m3)
    lo8 = (w32 - hi8.astype(np.float32)).astype(float8_e4m3)
    # j index on lhsT free = h*16 + r
    hi_t = hi8.view(np.uint8).transpose(4, 1, 3, 0, 2)            # p, q, k, h, r
    lo_t = lo8.view(np.uint8).transpose(4, 1, 3, 0, 2)
    wvt8 = np.zeros((P, 4, 5, 2, P), np.uint8)                    # p, q, blk, s, j
    for k in range(2):
        wvt8[:, :, k, 0] = hi_t[:, :, k].reshape(P, 4, P)
        wvt8[:, :, k, 1] = lo_t[:, :, k].reshape(P, 4, P)
    for pr2 in range(3):
        for two in range(2):
            wvt8[:, :, 2 + pr2, two] = hi_t[:, :, 2 + 2 * pr2 + two].reshape(P, 4, P)
    wvt8 = np.ascontiguousarray(wvt8.reshape(P, 5120))
    wo = f['wo']                                                  # (1024, 512)
    # wot[p, (m8*4+k4)*128 + j] = wo[128*m8+j, 128*k4+p]  (m-layout, pos path)
    wot = np.ascontiguousarray(
        wo.reshape(8, P, 4, P).transpose(3, 0, 2, 1).reshape(P, 4096)
    ).astype(bfloat16)
    # wotq[p2, (m8*4+q)*128 + j] = wo[128*m8+j, d(q,p2)], d = (p2//16)*64
    # + q*16 + p2%16  (q-permuted layout matching outcatV rows)
    wotq = np.ascontiguousarray(
        wo.reshape(8, P, 8, 4, 16).transpose(2, 4, 0, 3, 1).reshape(P, 4096)
    ).astype(bfloat16)
    # The transpose feeds pT rows n' = wc col index (0:512):
    #   n' in 0:392  -> spatial weight W~full[., n'] (own-window = p_t +
    #                   pm196; other-window = pm196)
    #   n' == 392    -> pm196
    #   n' in 393:512 -> zeros
    # Matching PV lhsT rows: n' < 392 -> PV[1 + n'%196]; n' == 392 ->
    # 196*PV[0] - 2*sum_t PV[1+t] (compensates the pm196 contamination and
    # adds the mean-token positional value p0*PV[0] = 196*pm196*PV[0]);
    # else 0.  pvt4 [128, (ci*4+m)*192 + .]: per (ci, m): [even-head 64 |
    # zeros 64 | odd-head 64] (odd lhsT = 128-wide slice from +64 so its
    # rows land at 64:128 with base partition 0).
    pvrows = np.zeros((512, 512), np.float32)
    for c in range(392):
        pvrows[c] = PV[1 + c % SP]
    pvrows[392] = SP * PV[0] - 2.0 * PV[1:NT].sum(axis=0)
    pvt4 = np.zeros((P, 3072), np.float32)
    for ci in range(4):
        for m in range(4):
            off = (ci * 4 + m) * 192
            blk = pvrows[ci * P:(ci + 1) * P]
            pvt4[:, off:off + 64] = blk[:, (2 * m) * 64:(2 * m + 1) * 64]
            pvt4[:, off + 128:off + 192] = blk[:, (2 * m + 1) * 64:(2 * m + 2) * 64]
    pvt4 = pvt4.astype(bfloat16)
    # mask: 1 on valid score cells [48, 392]
    # sel [48, 4*128]: one broadcast per jj serving all 4 V passes (the
    # q-permuted V layout has head p//16 on partition p):
    # sel[RB[jj] + p//16, jj*128 + p] = 1
    mask = np.zeros((48, 2 * SP), np.float32)
    sel = np.zeros((48, 4 * P), np.float32)
    for jj in range(4):
        jp = jj % 2
        mask[RB[jj]:RB[jj] + H, jp * SP:(jp + 1) * SP] = 1.0
        for p in range(P):
            sel[RB[jj] + p // 16, jj * P + p] = 1.0
    sel = sel.astype(bfloat16)
    ident = np.eye(P, dtype=np.float32)

    shared = {'wvt8': wvt8.view(float8_e4m3), 'wot': wot, 'wotq': wotq,
              'pvt4': pvt4, 'mask': mask, 'sel': sel, 'ident': ident,
              'ident16': ident.astype(bfloat16)}

    def per_core(bs, be):
        Bc = be - bs
        SG = Bc // SGB
        gd = np.zeros((SG, P, GD_COLS), np.uint8)
        # fm block: [p, k2*1568 + two*784 + jj*196 + t] = fm[bs+4g+jj, 128*(2k2+two)+p, t]
        gd[:, :, 0:FM_COLS] = (
            fm8[bs:be].reshape(SG, SGB, 4, 2, P, SP).transpose(0, 4, 2, 3, 1, 5)
            .reshape(SG, P, FM_COLS))
        # qt blocks: pair0 [p, QT_OFF + k2*32 + two*16 + 8*jp + h] (16-wide);
        # pair1 [p, QT_OFF+128 + k2*96 + two*48 + 32 + 8*jp + h] (48-wide,
        # zeros below q=32 so its scores land at psum rows 32:48 with base 0)
        qt6 = (Q_SCALE * Qt[bs:be]).reshape(SG, 2, 2, 4, 2, P, H)  # g,pair,jp,k2,two,p,h
        qtb0 = np.zeros((SG, 4, 2, P, 16), np.float32)
        qtb0[:, :, :, :, 0:H] = qt6[:, 0, 0]
        qtb0[:, :, :, :, 8:8 + H] = qt6[:, 0, 1]
        gd[:, :, QT_OFF:QT_OFF + 128] = _f8(
            qtb0.transpose(0, 3, 1, 2, 4).reshape(SG, P, 128))
        qtb1 = np.zeros((SG, 4, 2, P, 48), np.float32)
        qtb1[:, :, :, :, 32:32 + H] = qt6[:, 1, 0]
        qtb1[:, :, :, :, 40:40 + H] = qt6[:, 1, 1]
        gd[:, :, QT_OFF + 128:S2_OFF] = _f8(
            qtb1.transpose(0, 3, 1, 2, 4).reshape(SG, P, 384))
        # s2x block rows 0:48 (raw f32): additive scores with -1e30 masking;
        # col 392 = mean-token positional score.
        s2_4 = s2[bs:be].reshape(SG, SGB, NT, H).transpose(0, 1, 3, 2)   # g,jj,h,n
        s2x = np.full((SG, 48, 2 * SP + 1), -30000.0, np.float32)
        for jj in range(4):
            jp = jj % 2
            s2x[:, RB[jj]:RB[jj] + H, jp * SP:(jp + 1) * SP] = s2_4[:, jj, :, 1:]
            s2x[:, RB[jj]:RB[jj] + H, 2 * SP] = s2_4[:, jj, :, 0]
        gd[:, 0:48, S2_OFF:GD_COLS] = s2x.view(np.uint8).reshape(SG, 48, -1)
        # qT[p, m*Bc + b] = (queries + bo)[bs + b, 128m + p]
        qT = np.ascontiguousarray(
            qpb[bs:be].T.reshape(8, P, Bc).transpose(1, 0, 2).reshape(P, 8 * Bc))
        grep = np.ascontiguousarray(np.broadcast_to(f['ln_g'], (Bc, C)))
        brep = np.ascontiguousarray(np.broadcast_to(f['ln_b'], (Bc, C)))
        return {'gdata': np.ascontiguousarray(gd).view(float8_e4m3), 'qT': qT,
                'grep': grep, 'brep': brep, **shared}

    return per_core


# ---------------------------------------------------------------- bass build

def build_bass(G=16, debug=False, stage=3):
    import concourse.bacc as bacc
    import concourse.mybir as mybir
    import concourse.tile as tile

    f32 = mybir.dt.float32
    bf16 = mybir.dt.bfloat16
    f8 = mybir.dt.float8e4
    DR = mybir.MatmulPerfMode.DoubleRow
    Ax = mybir.AxisListType
    Op = mybir.AluOpType
    AF = mybir.ActivationFunctionType

    Bc = 2 * G
    SG = Bc // SGB
    nc = bacc.Bacc(trn_type="TRN2", name="mhma_v3")

    gd_d = nc.dram_tensor('gdata', (SG, P, GD_COLS), f8, kind="ExternalInput")
    wvt_d = nc.dram_tensor('wvt8', (P, 5120), f8, kind="ExternalInput")
    wot_d = nc.dram_tensor('wot', (P, 4096), bf16, kind="ExternalInput")
    wotq_d = nc.dram_tensor('wotq', (P, 4096), bf16, kind="ExternalInput")
    pvt_d = nc.dram_tensor('pvt4', (P, 3072), bf16, kind="ExternalInput")
    mask_d = nc.dram_tensor('mask', (48, 2 * SP), f32, kind="ExternalInput")
    sel_d = nc.dram_tensor('sel', (48, 4 * P), bf16, kind="ExternalInput")
    qT_d = nc.dram_tensor('qT', (P, 8 * Bc), f32, kind="ExternalInput")
    grep_d = nc.dram_tensor('grep', (Bc, C), f32, kind="ExternalInput")
    brep_d = nc.dram_tensor('brep', (Bc, C), f32, kind="ExternalInput")
    ident_d = nc.dram_tensor('ident', (P, P), f32, kind="ExternalInput")
    ident16_d = nc.dram_tensor('ident16', (P, P), bf16, kind="ExternalInput")
    out_d = nc.dram_tensor('out', (Bc, C), f32, kind="ExternalOutput")
    if debug:
        dbg_wc_d = nc.dram_tensor('dbg_wc', (48, 512), f32, kind="ExternalOutput")
        dbg_ps_d = nc.dram_tensor('dbg_ps', (48, 2 * SP), f32, kind="ExternalOutput")
        dbg_ocv_d = nc.dram_tensor('dbg_ocv', (P, 4 * Bc), f32, kind="ExternalOutput")
        dbg_pos_d = nc.dram_tensor('dbg_pos', (P, 128), f32, kind="ExternalOutput")
        dbg_pt_d = nc.dram_tensor('dbg_pt', (P, 2048), f32, kind="ExternalOutput")

    with tile.TileContext(nc) as tc:
        with tc.tile_pool(name="const", bufs=1) as cpool:
            # loop constants first (small before big) so gdata(0) starts
            # early; tail-only constants are DMA'd after the loop is issued.
            mask_sb = cpool.tile([48, 2 * SP], f32)
            sel_sb = cpool.tile([48, 4 * P], bf16)
            ident16_sb = cpool.tile([P, P], bf16)
            wvt_sb = cpool.tile([P, 5120], f8)
            nc.scalar.dma_start(out=wvt_sb[:, :], in_=wvt_d[:, :])
            wot_sb = cpool.tile([P, 4096], bf16)
            wotq_sb = cpool.tile([P, 4096], bf16)
            pvt_sb = cpool.tile([P, 3072], bf16)
            qT_sb = cpool.tile([P, 8 * Bc], f32)
            grep_sb = cpool.tile([Bc, C], f32)
            brep_sb = cpool.tile([Bc, C], f32)
            ident_sb = cpool.tile([P, P], f32)
            ones_sb = cpool.tile([P, 2], f32)
            nc.vector.memset(ones_sb[:, :], 1.0)
            sqwarm_sb = cpool.tile([1, 1], f32)
            nc.scalar.activation(sqwarm_sb[:, :], ones_sb[0:1, 0:1], AF.Sqrt)
            pTcat_sb = cpool.tile([P, 2048], bf16)
            outcatV_sb = cpool.tile([P, 4 * Bc], f32)
            outcat_sb = cpool.tile([P, 4 * Bc], bf16)

            with (
                tc.tile_pool(name="gd", bufs=3) as gd_pool,
                tc.tile_pool(name="soft", bufs=2) as soft,
                tc.tile_pool(name="wc", bufs=2) as wc_pool,
                tc.tile_pool(name="vsb", bufs=4) as v_pool,
                tc.tile_pool(name="junk", bufs=2) as junk_pool,
                tc.tile_pool(name="ps_s", bufs=2, space="PSUM") as ps_pool,
                tc.tile_pool(name="ps_v", bufs=2, space="PSUM") as pv_pool,
                tc.tile_pool(name="ps_w", bufs=2, space="PSUM") as pw_pool,
                tc.tile_pool(name="ps_tr", bufs=2, space="PSUM") as tr_pool,
            ):
                for g in range(SG):
                    gt = gd_pool.tile([P, GD_COLS], f8, tag="gd")
                    nc.sync.dma_start(out=gt[:, :], in_=gd_d[g])
                    if g == 0:
                        nc.sync.dma_start(out=mask_sb[:, :], in_=mask_d[:, :])
                        nc.sync.dma_start(out=sel_sb[:, :], in_=sel_d[:, :])
                        nc.sync.dma_start(out=ident16_sb[:, :], in_=ident16_d[:, :])
                    s2x = gt[0:48, S2_OFF:GD_COLS].bitcast(f32)

                    def fm_rhs(pair, k2):
                        return gt[:, k2 * 1568:(k2 + 1) * 1568].rearrange(
                            "p (two t) -> p two t", two=2)[:, :, pair * 392:(pair + 1) * 392]

                    # ---- scores: psum [48, 392], fp8 DoubleRow.
                    # pair1 first (48-wide lhsT with zeros on rows 0:32,
                    # start zeroes the whole [48, 392] region), pair0
                    # accumulates into rows 0:16.
                    ps_s_full = ps_pool.tile([P, 2 * SP], f32, tag="ps_s")
                    for k2 in range(4):
                        lhs = gt[:, QT_OFF + 128 + k2 * 96:
                                 QT_OFF + 128 + (k2 + 1) * 96].rearrange(
                                     "p (two q) -> p two q", two=2)
                        nc.tensor.matmul(
                            ps_s_full[0:48, :], lhs, fm_rhs(1, k2),
                            start=(k2 == 0), stop=False,
                            perf_mode=DR, skip_group_check=True)
                    for k2 in range(4):
                        lhs = gt[:, QT_OFF + k2 * 32:
                                 QT_OFF + (k2 + 1) * 32].rearrange(
                                     "p (two q) -> p two q", two=2)
                        nc.tensor.matmul(
                            ps_s_full[0:16, :], lhs, fm_rhs(0, k2),
                            start=False, stop=(k2 == 3),
                            perf_mode=DR, skip_group_check=True)

                    # ---- softmax on [48, *]; Q_SCALE descale via STT scalar
                    sc = soft.tile([48, 2 * SP], f32, tag="sc")
                    nc.vector.scalar_tensor_tensor(
                        out=sc[:, :], in0=ps_s_full[0:48, :], scalar=1.0 / Q_SCALE,
                        in1=s2x[:, 0:2 * SP], op0=Op.mult, op1=Op.add)
                    smp = soft.tile([48, 1], f32, tag="smp")
                    junk40 = soft.tile([48, 2 * SP], f32, tag="junk40")
                    nc.vector.scalar_tensor_tensor(
                        out=junk40[:, :], in0=ps_s_full[0:48, :], scalar=1.0 / Q_SCALE,
                        in1=mask_sb[:, :], op0=Op.mult, op1=Op.mult,
                        accum_out=smp[:, :])
                    smean = soft.tile([48, 1], f32, tag="smean")
                    nc.vector.tensor_scalar(
                        out=smean[:, :], in0=smp[:, :],
                        scalar1=1.0 / SP, scalar2=s2x[:, 2 * SP:2 * SP + 1],
                        op0=Op.mult, op1=Op.add)
                    mx1 = soft.tile([48, 1], f32, tag="mx1")
                    nc.vector.tensor_reduce(mx1[:, :], sc[:, :], Ax.X, Op.max)
                    nmx = soft.tile([48, 1], f32, tag="nmx")
                    nc.vector.tensor_scalar(
                        out=nmx[:, :], in0=mx1[:, :],
                        scalar1=smean[:, 0:1], scalar2=-1.0,
                        op0=Op.max, op1=Op.mult)
                    esc = soft.tile([48, 2 * SP], f32, tag="esc")
                    escs = soft.tile([48, 1], f32, tag="escs")
                    nc.scalar.activation(esc[:, :], sc[:, :], AF.Exp,
                                         bias=nmx[:, 0:1], scale=1.0,
                                         accum_out=escs[:, :])
                    emean = soft.tile([48, 1], f32, tag="emean")
                    nc.scalar.activation(emean[:, :], smean[:, :], AF.Exp,
                                         bias=nmx[:, 0:1], scale=1.0)
                    den = soft.tile([48, 1], f32, tag="den")
                    nc.vector.tensor_add(den[:, :], escs[:, :], emean[:, :])
                    rec = soft.tile([48, 1], f32, tag="rec")
                    nc.vector.reciprocal(rec[:, :], den[:, :])
                    pm196 = soft.tile([48, 1], f32, tag="pm196")
                    nc.vector.tensor_scalar(
                        out=pm196[:, :], in0=emean[:, :],
                        scalar1=rec[:, 0:1], scalar2=1.0 / SP,
                        op0=Op.mult, op1=Op.mult)
                    # wc layout [48, 512] bf16:
                    #   cols 0:392 = W~full = esc*rec + pm196 (own window =
                    #     spatial weights incl +p0/196; other window = pm196,
                    #     killed by selector zeros / compensated in pvt4)
                    #   col 392 = pm196; cols 393:512 = zeros
                    wc = wc_pool.tile([48, 512], bf16, tag="wc")
                    nc.vector.memset(wc[:, 393:512], 0.0)
                    nc.vector.tensor_copy(wc[0:48, 392:393], pm196[:, :])
                    for pr in range(2):
                        rs = slice(0, 32) if pr == 0 else slice(32, 48)
                        nc.vector.tensor_scalar(
                            out=wc[rs, 0:2 * SP], in0=esc[rs, :],
                            scalar1=rec[rs, 0:1], scalar2=pm196[rs, 0:1],
                            op0=Op.mult, op1=Op.add)
                    if debug and g == 0:
                        dbg_wc_sb = cpool.tile([48, 512], f32)
                        nc.vector.tensor_copy(dbg_wc_sb[:, :], wc[:, :])
                        nc.sync.dma_start(out=dbg_wc_d[:, :], in_=dbg_wc_sb[:, :])
                        dbg_ps_sb = cpool.tile([48, 2 * SP], f32)
                        nc.vector.tensor_copy(dbg_ps_sb[:, :], ps_s_full[0:48, :])
                        nc.sync.dma_start(out=dbg_ps_d[:, :], in_=dbg_ps_sb[:, :])

                    if g == SG - 1:
                        # last supergroup: transposes before V so the PE
                        # hides them under the V matmuls instead of
                        # stalling on the Act copy drain at loop exit.
                        for ci in range(4):
                            ps_tr = tr_pool.tile([P, 48], bf16, tag="ps_tr")
                            nc.tensor.transpose(
                                ps_tr[:, :], wc[0:48, ci * P:(ci + 1) * P],
                                ident16_sb[0:48, 0:48])
                            nc.scalar.copy(
                                pTcat_sb[:, ci * 512 + g * 64:ci * 512 + g * 64 + 48],
                                ps_tr[:, :])

                    # ---- V (fp8 DR, hi/lo slots) + per-jj selector broadcast
                    # + weighted sums (STT reads both psums, descale folded)
                    for pair in range(2):
                        pw = pw_pool.tile([P, 2 * SP], f32, tag="ps_w")
                        for jp in range(2):
                            jj = pair * 2 + jp
                            nc.tensor.matmul(
                                pw[:, jp * SP:(jp + 1) * SP],
                                sel_sb[32 * (jj // 2):32 * (jj // 2) + 16,
                                       jj * P:(jj + 1) * P],
                                wc[32 * (jj // 2):32 * (jj // 2) + 16,
                                   (jj % 2) * SP:(jj % 2 + 1) * SP],
                                start=True, stop=True)

                        for q in range(4):
                            ps_v = pv_pool.tile([P, 2 * SP], f32, tag="ps_v")
                            for k in range(2):
                                base = (k % 2) * 784 + pair * 392
                                rhs = gt[:, base:base + 392].rearrange(
                                    "p (one t) -> p one t", one=1
                                ).broadcast_to((P, 2, 392))
                                nc.tensor.matmul(
                                    ps_v[:, :],
                                    wvt_sb[:, (q * 5 + k) * 256:
                                           (q * 5 + k + 1) * 256].rearrange(
                                               "p (s j) -> p s j", s=2),
                                    rhs,
                                    start=(k == 0), stop=False,
                                    perf_mode=DR)
                            for pr2 in range(3):
                                nc.tensor.matmul(
                                    ps_v[:, :],
                                    wvt_sb[:, (q * 5 + 2 + pr2) * 256:
                                           (q * 5 + 3 + pr2) * 256].rearrange(
                                               "p (s j) -> p s j", s=2),
                                    fm_rhs(pair, 1 + pr2),
                                    start=False, stop=(pr2 == 2),
                                    perf_mode=DR)
                            v_sb = v_pool.tile([P, 2 * SP], f32, tag="vsb")
                            nc.scalar.activation(v_sb[:, :], ps_v[:, :], AF.Copy,
                                                 scale=1.0 / V_SCALE)
                            for jp in range(2):
                                b = SGB * g + pair * 2 + jp
                                junk = junk_pool.tile([P, SP], f32, tag="junk")
                                nc.vector.scalar_tensor_tensor(
                                    out=junk[:, :],
                                    in0=v_sb[:, jp * SP:(jp + 1) * SP],
                                    scalar=1.0, in1=pw[:, jp * SP:(jp + 1) * SP],
                                    op0=Op.mult, op1=Op.mult,
                                    accum_out=outcatV_sb[:, q * Bc + b:q * Bc + b + 1])

                    if g < SG - 1:
                        # pT transposes on the PE (ap=48, end of body) + Act
                        # psum->SBUF copies.
                        for ci in range(4):
                            ps_tr = tr_pool.tile([P, 48], bf16, tag="ps_tr")
                            nc.tensor.transpose(
                                ps_tr[:, :], wc[0:48, ci * P:(ci + 1) * P],
                                ident16_sb[0:48, 0:48])
                            nc.scalar.copy(
                                pTcat_sb[:, ci * 512 + g * 64:ci * 512 + g * 64 + 48],
                                ps_tr[:, :])

            # tail-only constants: issued after the loop so they queue
            # behind the gdata loads instead of delaying them.
            nc.sync.dma_start(out=wot_sb[:, :], in_=wot_d[:, :])
            nc.sync.dma_start(out=wotq_sb[:, :], in_=wotq_d[:, :])
            nc.sync.dma_start(out=pvt_sb[:, :], in_=pvt_d[:, :])
            nc.sync.dma_start(out=qT_sb[:, :], in_=qT_d[:, :])
            nc.sync.dma_start(out=grep_sb[:, :], in_=grep_d[:, :])
            nc.sync.dma_start(out=brep_sb[:, :], in_=brep_d[:, :])
            nc.sync.dma_start(out=ident_sb[:, :], in_=ident_d[:, :])

            # ---- tail: positional values, wo projection, LN, transpose
            with (
                tc.tile_pool(name="ps_pos", bufs=1, space="PSUM") as pos_pool,
                tc.tile_pool(name="ps_wo", bufs=2, space="PSUM") as wo_pool,
                tc.tile_pool(name="ps_st", bufs=1, space="PSUM") as st_pool,
                tc.tile_pool(name="ps_t", bufs=1, space="PSUM") as pt_pool,
                tc.tile_pool(name="tail", bufs=1) as tail_pool,
            ):
                # positional value sums: out[128d(m), 32b] += PVT_h @ pT_h.
                # Odd head first (128-wide lhsT, zeros in cols 0:64, start
                # zeroes rows 0:128), even head accumulates into rows 0:64.
                pos_ps = pos_pool.tile([P, 4 * 32], f32)  # [128, 32] per m-tile
                for m in range(4):
                    for ci in range(4):
                        h = 2 * m + 1
                        rhs = pTcat_sb[:, ci * 512:(ci + 1) * 512].rearrange(
                            "p (sg pb q r) -> p sg pb q r",
                            sg=8, pb=2, q=4)[:, :, :, 0:2, h:h + 1]
                        nc.tensor.matmul(
                            pos_ps[0:128, m * 32:(m + 1) * 32],
                            pvt_sb[:, (ci * 4 + m) * 192 + 64:(ci * 4 + m) * 192 + 192],
                            rhs,
                            start=(ci == 0), stop=False,
                            skip_group_check=True)
                    for ci in range(4):
                        h = 2 * m
                        rhs = pTcat_sb[:, ci * 512:(ci + 1) * 512].rearrange(
                            "p (sg pb q r) -> p sg pb q r",
                            sg=8, pb=2, q=4)[:, :, :, 0:2, h:h + 1]
                        nc.tensor.matmul(
                            pos_ps[0:64, m * 32:(m + 1) * 32],
                            pvt_sb[:, (ci * 4 + m) * 192:(ci * 4 + m) * 192 + 64],
                            rhs,
                            start=False, stop=(ci == 3),
                            skip_group_check=True)
                if debug:
                    nc.sync.dma_start(out=dbg_ocv_d[:, :], in_=outcatV_sb[:, :])
                    dbg_pos_sb = tail_pool.tile([P, 128], f32)
                    nc.vector.tensor_copy(dbg_pos_sb[:, :], pos_ps[:, :])
                    nc.sync.dma_start(out=dbg_pos_d[:, :], in_=dbg_pos_sb[:, :])
                nc.vector.tensor_copy(outcat_sb[:, :], outcatV_sb[:, :])
                pos_bf = tail_pool.tile([P, 128], bf16)
                nc.vector.tensor_copy(pos_bf[:, :], pos_ps[:, :])

                res_sb = tail_pool.tile([P, 8 * Bc], f32)
                ps_t = pt_pool.tile([Bc, C], f32)
                r2_sb = tail_pool.tile([P, Bc], f32)
                stat0 = st_pool.tile([Bc, 2], f32)
                stat1 = st_pool.tile([Bc, 2], f32)
                for m8 in range(8):
                    ps_wo = wo_pool.tile([P, Bc], f32, tag="ps_wo")
                    for q in range(4):
                        nc.tensor.matmul(
                            ps_wo[:, :],
                            wotq_sb[:, (m8 * 4 + q) * P:(m8 * 4 + q + 1) * P],
                            outcat_sb[:, q * Bc:(q + 1) * Bc],
                            start=(q == 0), stop=False)
                    for k4 in range(4):
                        nc.tensor.matmul(
                            ps_wo[:, :],
                            wot_sb[:, (m8 * 4 + k4) * P:(m8 * 4 + k4 + 1) * P],
                            pos_bf[:, k4 * 32:(k4 + 1) * 32],
                            start=False, stop=(k4 == 3))
                    r_m = res_sb[:, m8 * Bc:(m8 + 1) * Bc]
                    nc.vector.tensor_add(r_m, ps_wo[:, :], qT_sb[:, m8 * Bc:(m8 + 1) * Bc])
                    nc.scalar.square(r2_sb[:, :], r_m)
                    nc.tensor.matmul(stat0[:, :], r_m, ones_sb[:, :],
                                     start=(m8 == 0), stop=(m8 == 7),
                                     skip_group_check=True)
                    nc.tensor.matmul(stat1[:, :], r2_sb[:, :], ones_sb[:, :],
                                     start=(m8 == 0), stop=(m8 == 7),
                                     skip_group_check=True)
                    nc.tensor.transpose(
                        ps_t[:, m8 * P:(m8 + 1) * P],
                        res_sb[:, m8 * Bc:(m8 + 1) * Bc],
                        ident_sb[:, :])
                mean_sb = tail_pool.tile([Bc, 1], f32)
                nc.vector.tensor_scalar(out=mean_sb[:, :], in0=stat0[:, 0:1],
                                        scalar1=1.0 / C, scalar2=None, op0=Op.mult)
                ex2_sb = tail_pool.tile([Bc, 1], f32)
                nc.vector.tensor_scalar(out=ex2_sb[:, :], in0=stat1[:, 0:1],
                                        scalar1=1.0 / C, scalar2=None, op0=Op.mult)
                var_sb = tail_pool.tile([Bc, 1], f32)
                # var = ex2 - mean^2: (mean*mean - ex2) * -1
                nc.vector.scalar_tensor_tensor(
                    out=var_sb[:, :], in0=mean_sb[:, :], scalar=mean_sb[:, 0:1],
                    in1=ex2_sb[:, :], op0=Op.mult, op1=Op.subtract)
                nc.vector.tensor_scalar(out=var_sb[:, :], in0=var_sb[:, :],
                                        scalar1=-1.0, scalar2=None, op0=Op.mult)
                eps_sb = tail_pool.tile([Bc, 1], f32)
                nc.vector.memset(eps_sb[:, :], EPS)
                sd_sb = tail_pool.tile([Bc, 1], f32)
                nc.scalar.activation(sd_sb[:, :], var_sb[:, :], AF.Sqrt,
                                     bias=eps_sb[:, 0:1])
                rstd_sb = tail_pool.tile([Bc, 1], f32)
                nc.vector.reciprocal(rstd_sb[:, :], sd_sb[:, :])
                gr_sb = tail_pool.tile([Bc, C], f32)
                nc.vector.tensor_scalar(out=gr_sb[:, :], in0=grep_sb[:, :],
                                        scalar1=rstd_sb[:, 0:1], scalar2=None,
                                        op0=Op.mult)
                norm_sb = tail_pool.tile([Bc, C], f32)
                nc.vector.scalar_tensor_tensor(
                    out=norm_sb[:, :], in0=ps_t[:, :], scalar=mean_sb[:, 0:1],
                    in1=gr_sb[:, :], op0=Op.subtract, op1=Op.mult)
                fin_sb = tail_pool.tile([Bc, C], f32)
                nc.vector.tensor_add(fin_sb[:, :], norm_sb[:, :], brep_sb[:, :])
                nc.sync.dma_start(out=out_d[:, :], in_=fin_sb[:, :])

    nc.compile()
    return nc


# ---------------------------------------------------------------- entry

def kernel(**inputs):
    from concourse.bass_utils import run_bass_kernel_spmd

    per_core = _host_prep(inputs)
    B = inputs['feature_map'].shape[0]
    assert B == B_FULL, B
    bc = B // NCORES
    in_maps = [per_core(c * bc, (c + 1) * bc) for c in range(NCORES)]

    nc = build_bass(G=bc // 2)
    res = run_bass_kernel_spmd(nc, in_maps, core_ids=list(range(NCORES)))
    out = np.concatenate([r['out'] for r in res.results], axis=0)
    return out.astype(np.float32)
